# revision 37
# baseline (speedup 1.0000x reference)
"""Distributed GQA attention kernel for 8 TRN2 NeuronCores.

Problem: B=2, S=2048, DIM=2048, NH=32 q heads, NKV=8 kv heads, HD=64,
RoPE (base 10000), causal mask, out-projection.

Sharding (8 cores): core c -> batch b = c//4, rank r = c%4.
Each core handles 8 q heads (r*8 .. r*8+7) and their 2 kv heads (2r, 2r+1)
for its batch:
  - q/k/v projections column-sharded (head dim), x transposed on-chip.
  - RoPE on qT/kT.
  - Flash-style causal attention in transposed layout (scoresT [k, q]),
    softmax without max-subtraction (scores are O(5) here), sums via an
    extra ones-column on v.
  - Four chunked AllGathers of A^T (one per q-head pair tile, bf16) over
    the 4-core batch group, overlapped with attention on later pairs.
  - Each core out-projects its own S-slice (rank-dynamic column slice via
    the partition_id input) with the full Wo -> out [512, 2048].
Host reassembles: out[b, r*512:(r+1)*512, :] = core (4*b + r) output.

Head pairing trick: local q heads are stored as pairs (t, t+4) in each
128-partition tile so that a head's partition base (0 or 64) always equals
its kv head's partition base -> matmul operand bases match (walrus
requirement). Wq columns and Wo rows are permuted host-side to compensate.
"""

import os
import numpy as np

import concourse.bass as bass
import concourse.mybir as mybir
from concourse import bacc, tile
from concourse.bass import ds

F32 = mybir.dt.float32
F32R = mybir.dt.float32r
AF = mybir.ActivationFunctionType

# -------- problem constants (full size) --------
B, S, DIM = 2, 2048, 2048
NH, NKV, HD = 32, 8, 64
ROPE_BASE = 10000.0
N_CORES = 8
GROUP = 4                      # cores per batch group
NHL = NH // GROUP              # 8 local q heads
NKVL = NKV // GROUP            # 2 local kv heads
REP = NH // NKV                # 4 q heads per kv head
P = 128                        # partitions
NF = 512                       # free-dim tile (one PSUM bank of f32)

# results of the last device run (for test harness introspection)
LAST_RESULTS = None


def _rope_tables(S_, dtype=np.float32):
    """cos table tiled to 128 partitions, and a sign-folded sin table:
    rows p with p%64 < 32 carry -sin (x1 half), else +sin (x2 half)."""
    inv_freq = 1.0 / (ROPE_BASE ** (np.arange(0, HD, 2, dtype=np.float64) / HD))
    t = np.arange(S_, dtype=np.float64)
    freqs = inv_freq[:, None] * t[None, :]          # [32, S]
    cos32 = np.cos(freqs).astype(dtype)
    sin32 = np.sin(freqs).astype(dtype)
    c128 = np.tile(cos32, (4, 1))                   # [128, S]
    s128 = np.tile(np.concatenate([-sin32, sin32], axis=0), (2, 1))
    return c128, s128


def build_nc(S_=S, DIM_=DIM, NHL_=NHL, NKVL_=NKVL, group=GROUP, n_cores=N_CORES,
             mm="bf16"):
    """Build the SPMD per-core graph. All cores run the identical graph;
    per-core behavior comes only from the input shards (+ partition_id).

    mm: matmul compute dtype: "bf16" (fast weight loads, 1 cyc/row),
    "f32r" (4-byte, 1 cyc/row at N>=256 but slow weight path), "f32".
    """
    NQT = NHL_ // 2                  # q-head tiles of 128 (pairs)
    REP_ = NHL_ // NKVL_             # q heads per kv head (local)
    assert NKVL_ == 2, "kT/v1 layout assumes 2 local kv heads"
    DQ = NHL_ * HD                   # local q width (512)
    DKV = NKVL_ * HD                 # 128
    n_chunks = S_ // NF              # S chunks for projections
    d_tiles = DIM_ // P              # contraction tiles for projections
    kt_total = S_ // P               # key tiles
    n_qt = S_ // NF                  # q tiles per head (free dim NF)
    OUT_S = S_ // group              # output S-slice rows per core
    WO_R = group * DQ                # Wo rows (= NH*HD globally)
    WO_C = DIM_                      # output width
    BF16 = mybir.dt.bfloat16
    MMDT = {"bf16": BF16, "f32r": F32R, "f32": F32}[mm]
    # dtype used on the x-transpose path (must match matmul dtype for bf16)
    TDT = BF16 if mm == "bf16" else F32
    # for bf16 the host pre-converts inputs -> plain HWDGE loads, no SWDGE
    # cast DMAs clogging GpSimd
    host_bf16 = mm == "bf16"
    IN_DT = BF16 if host_bf16 else F32
    cast_w = (MMDT != F32) and not host_bf16

    nc = bacc.Bacc(None, target_bir_lowering=False, num_devices=n_cores)

    x_p = nc.declare_dram_parameter("x", [S_, DIM_], IN_DT, isOutput=False)
    wq_p = nc.declare_dram_parameter("wq", [DIM_, DQ], IN_DT, isOutput=False)
    wk_p = nc.declare_dram_parameter("wk", [DIM_, DKV], IN_DT, isOutput=False)
    wv_p = nc.declare_dram_parameter("wv", [DIM_, DKV], IN_DT, isOutput=False)
    wo_p = nc.declare_dram_parameter("wo", [WO_R, WO_C], IN_DT, isOutput=False)
    out_p = nc.declare_dram_parameter("out", [OUT_S, WO_C], F32, isOutput=True)

    import ml_dtypes
    np_tdt = ml_dtypes.bfloat16 if TDT == BF16 else np.float32
    cos_np, sin_np = _rope_tables(S_)
    cos_d = nc.inline_tensor(cos_np, name="cos_tab")
    sin_d = nc.inline_tensor(sin_np, name="sin_tab")
    ident_d = nc.inline_tensor(np.eye(P, dtype=np_tdt), name="ident")
    ones_d = nc.inline_tensor(np.ones((P, 1), dtype=np_tdt), name="ones_col")
    # causal 0/1 mask tiles for the 4 diagonal-block offsets
    np_mmdt = {F32: np.float32, F32R: np.float32,
               BF16: ml_dtypes.bfloat16}[MMDT]
    xx = np.arange(P)[:, None]
    yy = np.arange(NF)[None, :]
    mask_np = np.stack([(yy - xx - j * P >= 0) for j in range(NF // P)])
    # pairs of diagonal blocks share one [128, 2*NF] mask
    mask2_np = np.stack([np.concatenate([mask_np[j], mask_np[j + 1]], axis=1)
                         for j in (0, 2)])
    mask_d = nc.inline_tensor(mask2_np.astype(np_mmdt), name="cmask")

    groups = [list(range(g * group, (g + 1) * group))
              for g in range(n_cores // group)]

    def r(ap):
        if ap.dtype == MMDT or MMDT == F32:
            return ap
        return ap.bitcast(MMDT)

    with tile.TileContext(nc) as tc:
        # ---------------- persistent tiles ----------------
        with tc.tile_pool(name="persist", bufs=1) as pp:
            ident = pp.tile([P, P], TDT, name="ident_sb")
            id_dma = nc.gpsimd if (TDT != F32 and not host_bf16) else nc.sync
            id_dma.dma_start(out=ident, in_=ident_d[:, :])
            cmask = [pp.tile([P, 2 * NF], MMDT, name=f"cmask{j}")
                     for j in range(2)]
            for j in range(2):
                id_dma.dma_start(out=cmask[j], in_=mask_d[j, :, :])

            qT = [pp.tile([P, S_], MMDT, name=f"qT{t}") for t in range(NQT)]
            kT = pp.tile([P, S_], MMDT, name="kT")
            # v1 tiles: [128 keys, 130]: cols 0-63 kv0, 64 ones, 65-128 kv1,
            # 129 ones
            v1 = [pp.tile([P, 2 * (HD + 1)], MMDT, name=f"v1_{c}")
                  for c in range(kt_total)]

            # ---------------- phase 1: x^T + projections + rope ---------
            with tc.tile_pool(name="p1", bufs=1) as p1, \
                 tc.tile_pool(name="p1psum", bufs=2, space="PSUM") as pp1, \
                 tc.tile_pool(name="accpsum", bufs=1, space="PSUM") as pacc:
                c128 = p1.tile([P, S_], F32, name="c128")
                s128 = p1.tile([P, S_], F32, name="s128")
                nc.scalar.dma_start(out=c128, in_=cos_d[:, :])
                nc.scalar.dma_start(out=s128, in_=sin_d[:, :])
                wq_sb = [p1.tile([P, DQ], MMDT, name=f"wq{k}")
                         for k in range(d_tiles)]
                wk_sb = [p1.tile([P, DKV], MMDT, name=f"wk{k}")
                         for k in range(d_tiles)]
                wv_sb = [p1.tile([P, DKV], MMDT, name=f"wv{k}")
                         for k in range(d_tiles)]
                wdma = nc.gpsimd if cast_w else nc.sync
                for k in range(d_tiles):
                    wdma.dma_start(out=wq_sb[k], in_=wq_p[k * P:(k + 1) * P, :])
                    wdma.dma_start(out=wk_sb[k], in_=wk_p[k * P:(k + 1) * P, :])
                    wdma.dma_start(out=wv_sb[k], in_=wv_p[k * P:(k + 1) * P, :])

                vT = p1.tile([P, S_], TDT, name="vT")
                NM = NQT + 2
                xdma = nc.gpsimd if (TDT != F32 and not host_bf16) else nc.sync

                for n in range(n_chunks):
                    ncol = slice(n * NF, (n + 1) * NF)
                    # x rows for this chunk: [NF, DIM] as NF//P tiles
                    x_rows = []
                    for st in range(NF // P):
                        xr = p1.tile([P, DIM_], TDT, name="xrow", tag="xrow",
                                     bufs=2 * (NF // P))
                        xdma.dma_start(
                            out=xr,
                            in_=x_p[n * NF + st * P: n * NF + (st + 1) * P, :])
                        x_rows.append(xr)
                    # k-outer: transpose one d-tile, feed all NM accumulators
                    accs = [pacc.tile([P, NF], F32, name=f"acc{m}",
                                      tag=f"acc{m}") for m in range(NM)]
                    for k in range(d_tiles):
                        tp = pp1.tile([P, NF], TDT, name="tp", tag="tp")
                        for st in range(NF // P):
                            nc.tensor.transpose(
                                tp[:, st * P:(st + 1) * P],
                                x_rows[st][:, k * P:(k + 1) * P],
                                ident)
                        xt = p1.tile([P, NF], MMDT, name="xt", tag="xt",
                                     bufs=4)
                        nc.scalar.activation(xt, tp, AF.Copy)
                        for m in range(NM):
                            if m < NQT:
                                lhsT = wq_sb[k][:, m * P:(m + 1) * P]
                            elif m == NQT:
                                lhsT = wk_sb[k]
                            else:
                                lhsT = wv_sb[k]
                            nc.tensor.matmul(accs[m], r(lhsT), r(xt),
                                             start=(k == 0),
                                             stop=(k == d_tiles - 1))
                    for m in range(NM):
                        acc = accs[m]
                        if m <= NQT:
                            # rope: dest qT[m] or kT.
                            # All DVE ops must be partition-aligned, so
                            # build a half-swapped copy via SBUF DMAs and
                            # use the sign-folded sin table:
                            #   out = raw*cos + swap(raw)*sgn_sin
                            dest = qT[m] if m < NQT else kT
                            qraw = p1.tile([P, NF], F32, name="qraw",
                                           tag="qraw", bufs=2)
                            nc.scalar.activation(qraw, acc, AF.Copy)
                            qswp = p1.tile([P, NF], F32, name="qswp",
                                           tag="qswp", bufs=2)
                            h32 = HD // 2
                            for blk in range(4):
                                src = blk ^ 1      # swap 32-blocks in pairs
                                nc.sync.dma_start(
                                    out=qswp[blk * h32:(blk + 1) * h32, :],
                                    in_=qraw[src * h32:(src + 1) * h32, :])
                            tcos = p1.tile([P, NF], F32, name="tcos",
                                           tag="tcos", bufs=2)
                            tsin = p1.tile([P, NF], F32, name="tsin",
                                           tag="tsin", bufs=2)
                            nc.vector.tensor_mul(tcos, qraw, c128[:, ncol])
                            nc.vector.tensor_mul(tsin, qswp, s128[:, ncol])
                            nc.vector.tensor_add(dest[:, ncol], tcos, tsin)
                        else:
                            nc.scalar.activation(vT[:, ncol], acc, AF.Copy)

                # ---- v transform: vT [128, S] -> v1 tiles [keys, 130] ----
                for c in range(kt_total):
                    tpv = pp1.tile([P, P], TDT, name="tpv", tag="tp")
                    nc.tensor.transpose(tpv, vT[:, c * P:(c + 1) * P], ident)
                    nc.vector.tensor_copy(v1[c][:, 0:HD], tpv[:, 0:HD])
                    nc.vector.tensor_copy(v1[c][:, HD + 1:2 * HD + 1],
                                          tpv[:, HD:2 * HD])
                    wdma.dma_start(out=v1[c][:, HD:HD + 1], in_=ones_d[:, :])
                    wdma.dma_start(out=v1[c][:, 2 * HD + 1:2 * HD + 2],
                                   in_=ones_d[:, :])

            # comm bounce buffers: one AllGather per q-head pair tile so the
            # collective for pair t overlaps attention on pair t+1
            CDT = BF16 if mm == "bf16" else F32
            dram_pool = tc.tile_pool(name="dram", bufs=1, space="DRAM")
            dp = dram_pool.__enter__()
            ag_in = [dp.tile([P, S_], CDT, name=f"ag_in{t}")
                     for t in range(NQT)]
            ag_out = [dp.tile([group * P, S_], CDT, name=f"ag_out{t}")
                      for t in range(NQT)]

            # Wo + A^T tiles for the out-projection (prefetched during
            # attention; everything fits easily in bf16)
            p5_cm = tc.tile_pool(name="p5", bufs=1)
            p5 = p5_cm.__enter__()
            wo_sb = {}
            wo_pending = []
            for kidx in range(WO_R // P):
                for n_o in range(WO_C // NF):
                    w = p5.tile([P, NF], MMDT, name=f"wo{kidx}_{n_o}")
                    wo_sb[(kidx, n_o)] = w
                    wo_pending.append((kidx, n_o))
            wo_dma = nc.gpsimd if cast_w else nc.sync

            def drip_wo(count):
                # spread the Wo prefetch so the DMA queue never backs up
                for _ in range(min(count, len(wo_pending))):
                    kidx, n_o = wo_pending.pop(0)
                    wo_dma.dma_start(
                        out=wo_sb[(kidx, n_o)],
                        in_=wo_p[kidx * P:(kidx + 1) * P,
                                 n_o * NF:(n_o + 1) * NF])

            # ---------------- phase 3: attention + chunked AllGather ----
            with tc.tile_pool(name="p3", bufs=1) as p3, \
                 tc.tile_pool(name="scpsum", bufs=4, space="PSUM") as psc, \
                 tc.tile_pool(name="otpsum", bufs=4, space="PSUM") as pot:
                for t in range(NQT):
                    pair = [t, t + NQT] if NQT < NHL_ else [t]
                    for qi in range(n_qt):
                        nk = (qi + 1) * NF // P
                        otps = {}
                        for h in pair:
                            otps[h] = pot.tile([HD + 1, NF], F32,
                                               name="otp", tag="otp")
                        for kt in range(nk):
                            ats = {}
                            for h in pair:
                                kv = h // REP_
                                hb = (h // NQT) * HD
                                assert hb == kv * HD
                                sc = psc.tile([P, NF], F32, name="sc",
                                              tag="sc")
                                nc.tensor.matmul(
                                    sc,
                                    r(kT[hb:hb + HD, kt * P:(kt + 1) * P]),
                                    r(qT[t][hb:hb + HD,
                                            qi * NF:(qi + 1) * NF]),
                                    start=True, stop=True)
                                at = p3.tile([P, NF], MMDT, name="at",
                                             tag="at", bufs=6)
                                nc.scalar.activation(at, sc, AF.Exp,
                                                     scale=float(HD) ** -0.5)
                                if kt >= qi * NF // P:
                                    # zero future keys (0/1 causal mask,
                                    # sliced from the 2-block mask tiles)
                                    j = kt - qi * NF // P
                                    eng = nc.vector if j % 2 == 0 \
                                        else nc.gpsimd
                                    eng.tensor_mul(
                                        at, at,
                                        cmask[j // 2][:, (j % 2) * NF:
                                                      (j % 2 + 1) * NF])
                                ats[h] = at
                            for h in pair:
                                kv = h // REP_
                                nc.tensor.matmul(
                                    otps[h],
                                    r(v1[kt][:, kv * (HD + 1):
                                             (kv + 1) * (HD + 1)]),
                                    r(ats[h]),
                                    start=(kt == 0), stop=(kt == nk - 1))
                        for h in pair:
                            otp = otps[h]
                            # rec sliced at base 64 so reciprocal's in/out
                            # APs sit on the same partition
                            rec = p3.tile([HD + 1, NF], F32, name="rec",
                                          tag="rec", bufs=2)
                            nc.vector.reciprocal(rec[HD:HD + 1, :],
                                                 otp[HD:HD + 1, :])
                            # broadcast across partitions via a DRAM hop
                            # with a stride-0 partition read (keeps GpSimd
                            # out of the chain)
                            recd = dp.tile([1, NF], F32, name="recd",
                                           tag="recd", bufs=3)
                            nc.sync.dma_start(out=recd,
                                              in_=rec[HD:HD + 1, :])
                            bc = p3.tile([HD, NF], F32, name="bc", tag="bc",
                                         bufs=2)
                            rb_bcast = bass.AP(
                                tensor=recd.tensor, offset=recd.offset,
                                ap=[[0, HD]] + list(recd.ap)[1:])
                            nc.sync.dma_start(out=bc, in_=rb_bcast)
                            ao = p3.tile([HD, NF], CDT, name="ao", tag="ao",
                                         bufs=3)
                            nc.vector.tensor_mul(ao, otp[0:HD, :], bc)
                            off = (h // NQT) * HD
                            nc.sync.dma_start(
                                out=ag_in[t][off:off + HD,
                                             qi * NF:(qi + 1) * NF],
                                in_=ao)
                        drip_wo(4)
                    drip_wo(len(wo_pending) if t == NQT - 1 else 0)
                    # pair tile t complete -> AllGather it while the next
                    # pair computes
                    nc.gpsimd.collective_compute(
                        "AllGather", mybir.AluOpType.bypass,
                        replica_groups=groups,
                        ins=[ag_in[t].opt()], outs=[ag_out[t].opt()])

            # ---------------- phase 5: out projection ----------------
            with tc.tile_pool(name="oppsum", bufs=6, space="PSUM") as pop:
                # rank-dynamic S-slice columns
                pid = nc.sync.partition_id()
                col0 = (pid % group) * OUT_S
                atf = {}
                for t in range(NQT):
                    for g in range(group):
                        a = p5.tile([P, OUT_S], MMDT, name=f"atf{t}_{g}")
                        nc.sync.dma_start(
                            out=a,
                            in_=ag_out[t][g * P:(g + 1) * P,
                                          ds(col0, OUT_S)])
                        atf[(t, g)] = a

                kts = [(t, g) for t in range(NQT) for g in range(group)]
                for n_o in range(WO_C // NF):
                    for m in range(OUT_S // P):
                        op = pop.tile([P, NF], F32, name="op", tag="op")
                        for kidx, (t, g) in enumerate(kts):
                            nc.tensor.matmul(
                                op,
                                r(atf[(t, g)][:, m * P:(m + 1) * P]),
                                r(wo_sb[(kidx, n_o)]),
                                start=(kidx == 0),
                                stop=(kidx == len(kts) - 1))
                        osb = p5.tile([P, NF], F32, name="osb", tag="osb",
                                      bufs=3)
                        nc.scalar.activation(osb, op, AF.Copy)
                        nc.sync.dma_start(
                            out=out_p[m * P:(m + 1) * P,
                                      n_o * NF:(n_o + 1) * NF],
                            in_=osb)
            p5_cm.__exit__(None, None, None)
            dram_pool.__exit__(None, None, None)
    nc.finalize()
    return nc


# ---------------------------------------------------------------------------
# host-side sharding / unsharding
# ---------------------------------------------------------------------------

def _local_head_perm(nhl):
    """Local head order: pairs (t, t + nhl//2)."""
    nqt = nhl // 2
    order = []
    for t in range(nqt):
        order.append(t)
        order.append(t + nqt)
    return order


def shard_inputs(x, Wq, Wk, Wv, Wo, S_=S, nhl=NHL, nkvl=NKVL, group=GROUP,
                 n_cores=N_CORES, mm="bf16"):
    """Build per-core input maps (converted to bf16 for mm="bf16")."""
    import ml_dtypes
    dt_ = ml_dtypes.bfloat16 if mm == "bf16" else np.float32
    perm = _local_head_perm(nhl)
    in_maps = []
    # Wo rows permuted identically for all cores. Order must match the
    # device-side gathered A^T layout: for each pair-tile t, the AllGather
    # concatenates the 4 ranks' 128-row chunks; within a chunk the rows are
    # heads (t, t+nqt) x 64.
    nqt = nhl // 2
    row_idx = []
    for t in range(nqt):
        for g in range(group):
            for h in (g * nhl + t, g * nhl + t + nqt):
                row_idx.extend(range(h * HD, (h + 1) * HD))
    wo_perm = np.ascontiguousarray(Wo[row_idx, :]).astype(dt_)
    for c in range(n_cores):
        b, rk = c // group, c % group
        col_idx = []
        for t in perm:
            h = rk * nhl + t
            col_idx.extend(range(h * HD, (h + 1) * HD))
        kv_cols = []
        for kvh in range(rk * nkvl, (rk + 1) * nkvl):
            kv_cols.extend(range(kvh * HD, (kvh + 1) * HD))
        in_maps.append({
            "x": np.ascontiguousarray(x[b]).astype(dt_),
            "wq": np.ascontiguousarray(Wq[:, col_idx]).astype(dt_),
            "wk": np.ascontiguousarray(Wk[:, kv_cols]).astype(dt_),
            "wv": np.ascontiguousarray(Wv[:, kv_cols]).astype(dt_),
            "wo": wo_perm,
        })
    return in_maps


def unshard_output(results, S_=S, group=GROUP, n_cores=N_CORES, width=DIM):
    out = np.zeros((n_cores // group, S_, width), dtype=np.float32)
    sl = S_ // group
    for c in range(n_cores):
        b, rk = c // group, c % group
        out[b, rk * sl:(rk + 1) * sl, :] = results[c]["out"]
    return out


_NC_CACHE = {}


def kernel(x, mask=None, Wq=None, Wk=None, Wv=None, Wo=None):
    """Full-input entry point: returns [B, S, DIM] float32."""
    global LAST_RESULTS
    from concourse.bass_utils import run_bass_kernel_spmd

    x = np.asarray(x, dtype=np.float32)
    key = "full"
    if key not in _NC_CACHE:
        _NC_CACHE[key] = build_nc()
    nc = _NC_CACHE[key]
    in_maps = shard_inputs(x, np.asarray(Wq), np.asarray(Wk),
                           np.asarray(Wv), np.asarray(Wo))
    res = run_bass_kernel_spmd(nc, in_maps, core_ids=list(range(N_CORES)),
                               trace=bool(os.environ.get("KERNEL_TRACE")))
    LAST_RESULTS = res
    return unshard_output(res.results)


# revision 38
# speedup vs baseline: 1.0397x; 1.0397x over previous
"""Distributed GQA attention kernel for 8 TRN2 NeuronCores.

Problem: B=2, S=2048, DIM=2048, NH=32 q heads, NKV=8 kv heads, HD=64,
RoPE (base 10000), causal mask, out-projection.

Sharding (8 cores): core c -> batch b = c//4, rank r = c%4.
Each core handles 8 q heads (r*8 .. r*8+7) and their 2 kv heads (2r, 2r+1)
for its batch:
  - q/k/v projections column-sharded (head dim), x transposed on-chip.
  - RoPE on qT/kT.
  - Flash-style causal attention in transposed layout (scoresT [k, q]),
    softmax without max-subtraction (scores are O(5) here), sums via an
    extra ones-column on v.
  - Four chunked AllGathers of A^T (one per q-head pair tile, bf16) over
    the 4-core batch group, overlapped with attention on later pairs.
  - Each core out-projects its own S-slice (rank-dynamic column slice via
    the partition_id input) with the full Wo -> out [512, 2048].
Host reassembles: out[b, r*512:(r+1)*512, :] = core (4*b + r) output.

Head pairing trick: local q heads are stored as pairs (t, t+4) in each
128-partition tile so that a head's partition base (0 or 64) always equals
its kv head's partition base -> matmul operand bases match (walrus
requirement). Wq columns and Wo rows are permuted host-side to compensate.
"""

import os
import numpy as np

import concourse.bass as bass
import concourse.mybir as mybir
from concourse import bacc, tile
from concourse.bass import ds

F32 = mybir.dt.float32
F32R = mybir.dt.float32r
AF = mybir.ActivationFunctionType

# -------- problem constants (full size) --------
B, S, DIM = 2, 2048, 2048
NH, NKV, HD = 32, 8, 64
ROPE_BASE = 10000.0
N_CORES = 8
GROUP = 4                      # cores per batch group
NHL = NH // GROUP              # 8 local q heads
NKVL = NKV // GROUP            # 2 local kv heads
REP = NH // NKV                # 4 q heads per kv head
P = 128                        # partitions
NF = 512                       # free-dim tile (one PSUM bank of f32)

# results of the last device run (for test harness introspection)
LAST_RESULTS = None


def _rope_tables(S_, dtype=np.float32):
    """cos table tiled to 128 partitions, and a sign-folded sin table:
    rows p with p%64 < 32 carry -sin (x1 half), else +sin (x2 half)."""
    inv_freq = 1.0 / (ROPE_BASE ** (np.arange(0, HD, 2, dtype=np.float64) / HD))
    t = np.arange(S_, dtype=np.float64)
    freqs = inv_freq[:, None] * t[None, :]          # [32, S]
    cos32 = np.cos(freqs).astype(dtype)
    sin32 = np.sin(freqs).astype(dtype)
    c128 = np.tile(cos32, (4, 1))                   # [128, S]
    s128 = np.tile(np.concatenate([-sin32, sin32], axis=0), (2, 1))
    return c128, s128


def build_nc(S_=S, DIM_=DIM, NHL_=NHL, NKVL_=NKVL, group=GROUP, n_cores=N_CORES,
             mm="bf16"):
    """Build the SPMD per-core graph. All cores run the identical graph;
    per-core behavior comes only from the input shards (+ partition_id).

    mm: matmul compute dtype: "bf16" (fast weight loads, 1 cyc/row),
    "f32r" (4-byte, 1 cyc/row at N>=256 but slow weight path), "f32".
    """
    NQT = NHL_ // 2                  # q-head tiles of 128 (pairs)
    REP_ = NHL_ // NKVL_             # q heads per kv head (local)
    assert NKVL_ == 2, "kT/v1 layout assumes 2 local kv heads"
    DQ = NHL_ * HD                   # local q width (512)
    DKV = NKVL_ * HD                 # 128
    n_chunks = S_ // NF              # S chunks for projections
    d_tiles = DIM_ // P              # contraction tiles for projections
    kt_total = S_ // P               # key tiles
    n_qt = S_ // NF                  # q tiles per head (free dim NF)
    OUT_S = S_ // group              # output S-slice rows per core
    WO_R = group * DQ                # Wo rows (= NH*HD globally)
    WO_C = DIM_                      # output width
    BF16 = mybir.dt.bfloat16
    MMDT = {"bf16": BF16, "f32r": F32R, "f32": F32}[mm]
    # dtype used on the x-transpose path (must match matmul dtype for bf16)
    TDT = BF16 if mm == "bf16" else F32
    # for bf16 the host pre-converts inputs -> plain HWDGE loads, no SWDGE
    # cast DMAs clogging GpSimd
    host_bf16 = mm == "bf16"
    IN_DT = BF16 if host_bf16 else F32
    cast_w = (MMDT != F32) and not host_bf16

    nc = bacc.Bacc(None, target_bir_lowering=False, num_devices=n_cores)

    x_p = nc.declare_dram_parameter("x", [S_, DIM_], IN_DT, isOutput=False)
    wq_p = nc.declare_dram_parameter("wq", [DIM_, DQ], IN_DT, isOutput=False)
    wk_p = nc.declare_dram_parameter("wk", [DIM_, DKV], IN_DT, isOutput=False)
    wv_p = nc.declare_dram_parameter("wv", [DIM_, DKV], IN_DT, isOutput=False)
    wo_p = nc.declare_dram_parameter("wo", [WO_R, WO_C], IN_DT, isOutput=False)
    out_p = nc.declare_dram_parameter("out", [OUT_S, WO_C], F32, isOutput=True)

    import ml_dtypes
    np_tdt = ml_dtypes.bfloat16 if TDT == BF16 else np.float32
    cos_np, sin_np = _rope_tables(S_)
    cos_d = nc.inline_tensor(cos_np, name="cos_tab")
    sin_d = nc.inline_tensor(sin_np, name="sin_tab")
    ident_d = nc.inline_tensor(np.eye(P, dtype=np_tdt), name="ident")
    ones_d = nc.inline_tensor(np.ones((P, 1), dtype=np_tdt), name="ones_col")
    # causal 0/1 mask tiles for the 4 diagonal-block offsets
    np_mmdt = {F32: np.float32, F32R: np.float32,
               BF16: ml_dtypes.bfloat16}[MMDT]
    xx = np.arange(P)[:, None]
    yy = np.arange(NF)[None, :]
    mask_np = np.stack([(yy - xx - j * P >= 0) for j in range(NF // P)])
    # pairs of diagonal blocks share one [128, 2*NF] mask
    mask2_np = np.stack([np.concatenate([mask_np[j], mask_np[j + 1]], axis=1)
                         for j in (0, 2)])
    mask_d = nc.inline_tensor(mask2_np.astype(np_mmdt), name="cmask")

    groups = [list(range(g * group, (g + 1) * group))
              for g in range(n_cores // group)]

    def r(ap):
        if ap.dtype == MMDT or MMDT == F32:
            return ap
        return ap.bitcast(MMDT)

    with tile.TileContext(nc) as tc:
        # ---------------- persistent tiles ----------------
        with tc.tile_pool(name="persist", bufs=1) as pp:
            ident = pp.tile([P, P], TDT, name="ident_sb")
            id_dma = nc.gpsimd if (TDT != F32 and not host_bf16) else nc.sync
            id_dma.dma_start(out=ident, in_=ident_d[:, :])
            cmask = [pp.tile([P, 2 * NF], MMDT, name=f"cmask{j}")
                     for j in range(2)]
            for j in range(2):
                id_dma.dma_start(out=cmask[j], in_=mask_d[j, :, :])

            qT = [pp.tile([P, S_], MMDT, name=f"qT{t}") for t in range(NQT)]
            kT = pp.tile([P, S_], MMDT, name="kT")
            # v1 tiles: [128 keys, 130]: cols 0-63 kv0, 64 ones, 65-128 kv1,
            # 129 ones
            v1 = [pp.tile([P, 2 * (HD + 1)], MMDT, name=f"v1_{c}")
                  for c in range(kt_total)]

            # ---------------- phase 1: x^T + projections + rope ---------
            with tc.tile_pool(name="p1", bufs=1) as p1, \
                 tc.tile_pool(name="p1psum", bufs=2, space="PSUM") as pp1, \
                 tc.tile_pool(name="accpsum", bufs=1, space="PSUM") as pacc:
                c128 = p1.tile([P, S_], F32, name="c128")
                s128 = p1.tile([P, S_], F32, name="s128")
                nc.scalar.dma_start(out=c128, in_=cos_d[:, :])
                nc.scalar.dma_start(out=s128, in_=sin_d[:, :])
                wq_sb = [p1.tile([P, DQ], MMDT, name=f"wq{k}")
                         for k in range(d_tiles)]
                wk_sb = [p1.tile([P, DKV], MMDT, name=f"wk{k}")
                         for k in range(d_tiles)]
                wv_sb = [p1.tile([P, DKV], MMDT, name=f"wv{k}")
                         for k in range(d_tiles)]
                wdma = nc.gpsimd if cast_w else nc.sync
                for k in range(d_tiles):
                    wdma.dma_start(out=wq_sb[k], in_=wq_p[k * P:(k + 1) * P, :])
                    wdma.dma_start(out=wk_sb[k], in_=wk_p[k * P:(k + 1) * P, :])
                    wdma.dma_start(out=wv_sb[k], in_=wv_p[k * P:(k + 1) * P, :])

                vT = p1.tile([P, S_], TDT, name="vT")
                NM = NQT + 2
                xdma = nc.gpsimd if (TDT != F32 and not host_bf16) else nc.sync

                for n in range(n_chunks):
                    ncol = slice(n * NF, (n + 1) * NF)
                    # x rows for this chunk: [NF, DIM] as NF//P tiles
                    x_rows = []
                    for st in range(NF // P):
                        xr = p1.tile([P, DIM_], TDT, name="xrow", tag="xrow",
                                     bufs=2 * (NF // P))
                        xdma.dma_start(
                            out=xr,
                            in_=x_p[n * NF + st * P: n * NF + (st + 1) * P, :])
                        x_rows.append(xr)
                    # k-outer: transpose one d-tile, feed all NM accumulators
                    accs = [pacc.tile([P, NF], F32, name=f"acc{m}",
                                      tag=f"acc{m}") for m in range(NM)]
                    for k in range(d_tiles):
                        tp = pp1.tile([P, NF], TDT, name="tp", tag="tp")
                        for st in range(NF // P):
                            nc.tensor.transpose(
                                tp[:, st * P:(st + 1) * P],
                                x_rows[st][:, k * P:(k + 1) * P],
                                ident)
                        xt = p1.tile([P, NF], MMDT, name="xt", tag="xt",
                                     bufs=4)
                        nc.scalar.activation(xt, tp, AF.Copy)
                        for m in range(NM):
                            if m < NQT:
                                lhsT = wq_sb[k][:, m * P:(m + 1) * P]
                            elif m == NQT:
                                lhsT = wk_sb[k]
                            else:
                                lhsT = wv_sb[k]
                            nc.tensor.matmul(accs[m], r(lhsT), r(xt),
                                             start=(k == 0),
                                             stop=(k == d_tiles - 1))
                    for m in range(NM):
                        acc = accs[m]
                        if m <= NQT:
                            # rope: dest qT[m] or kT.
                            # All DVE ops must be partition-aligned, so
                            # build a half-swapped copy via SBUF DMAs and
                            # use the sign-folded sin table:
                            #   out = raw*cos + swap(raw)*sgn_sin
                            dest = qT[m] if m < NQT else kT
                            qraw = p1.tile([P, NF], F32, name="qraw",
                                           tag="qraw", bufs=2)
                            nc.scalar.activation(qraw, acc, AF.Copy)
                            qswp = p1.tile([P, NF], F32, name="qswp",
                                           tag="qswp", bufs=2)
                            h32 = HD // 2
                            for blk in range(4):
                                src = blk ^ 1      # swap 32-blocks in pairs
                                nc.sync.dma_start(
                                    out=qswp[blk * h32:(blk + 1) * h32, :],
                                    in_=qraw[src * h32:(src + 1) * h32, :])
                            tcos = p1.tile([P, NF], F32, name="tcos",
                                           tag="tcos", bufs=2)
                            tsin = p1.tile([P, NF], F32, name="tsin",
                                           tag="tsin", bufs=2)
                            nc.vector.tensor_mul(tcos, qraw, c128[:, ncol])
                            nc.vector.tensor_mul(tsin, qswp, s128[:, ncol])
                            nc.vector.tensor_add(dest[:, ncol], tcos, tsin)
                        else:
                            nc.scalar.activation(vT[:, ncol], acc, AF.Copy)

                # ---- v transform: vT [128, S] -> v1 tiles [keys, 130] ----
                for c in range(kt_total):
                    tpv = pp1.tile([P, P], TDT, name="tpv", tag="tp")
                    nc.tensor.transpose(tpv, vT[:, c * P:(c + 1) * P], ident)
                    nc.vector.tensor_copy(v1[c][:, 0:HD], tpv[:, 0:HD])
                    nc.vector.tensor_copy(v1[c][:, HD + 1:2 * HD + 1],
                                          tpv[:, HD:2 * HD])
                    wdma.dma_start(out=v1[c][:, HD:HD + 1], in_=ones_d[:, :])
                    wdma.dma_start(out=v1[c][:, 2 * HD + 1:2 * HD + 2],
                                   in_=ones_d[:, :])

            # comm bounce buffers: one AllGather per q-head pair tile so the
            # collective for pair t overlaps attention on pair t+1
            CDT = BF16 if mm == "bf16" else F32
            dram_pool = tc.tile_pool(name="dram", bufs=1, space="DRAM")
            dp = dram_pool.__enter__()
            ag_in = [dp.tile([P, S_], CDT, name=f"ag_in{t}")
                     for t in range(NQT)]
            ag_out = [dp.tile([group * P, S_], CDT, name=f"ag_out{t}")
                      for t in range(NQT)]

            # Wo + A^T tiles for the out-projection (prefetched during
            # attention; everything fits easily in bf16)
            p5_cm = tc.tile_pool(name="p5", bufs=1)
            p5 = p5_cm.__enter__()
            wo_sb = {}
            wo_pending = []
            for kidx in range(WO_R // P):
                for n_o in range(WO_C // NF):
                    w = p5.tile([P, NF], MMDT, name=f"wo{kidx}_{n_o}")
                    wo_sb[(kidx, n_o)] = w
                    wo_pending.append((kidx, n_o))
            wo_dma = nc.gpsimd if cast_w else nc.sync

            def drip_wo(count):
                # spread the Wo prefetch so the DMA queue never backs up
                for _ in range(min(count, len(wo_pending))):
                    kidx, n_o = wo_pending.pop(0)
                    wo_dma.dma_start(
                        out=wo_sb[(kidx, n_o)],
                        in_=wo_p[kidx * P:(kidx + 1) * P,
                                 n_o * NF:(n_o + 1) * NF])

            # ---------------- phase 3: attention + chunked AllGather ----
            with tc.tile_pool(name="p3", bufs=1) as p3, \
                 tc.tile_pool(name="scpsum", bufs=4, space="PSUM") as psc, \
                 tc.tile_pool(name="otpsum", bufs=4, space="PSUM") as pot:
                for t in range(NQT):
                    pair = [t, t + NQT] if NQT < NHL_ else [t]
                    for qi in range(n_qt):
                        nk = (qi + 1) * NF // P
                        otps = {}
                        for h in pair:
                            otps[h] = pot.tile([HD + 1, NF], F32,
                                               name="otp", tag="otp")
                        for kt in range(nk):
                            ats = {}
                            for h in pair:
                                kv = h // REP_
                                hb = (h // NQT) * HD
                                assert hb == kv * HD
                                sc = psc.tile([P, NF], F32, name="sc",
                                              tag="sc")
                                nc.tensor.matmul(
                                    sc,
                                    r(kT[hb:hb + HD, kt * P:(kt + 1) * P]),
                                    r(qT[t][hb:hb + HD,
                                            qi * NF:(qi + 1) * NF]),
                                    start=True, stop=True)
                                at = p3.tile([P, NF], MMDT, name="at",
                                             tag="at", bufs=6)
                                nc.scalar.activation(at, sc, AF.Exp,
                                                     scale=float(HD) ** -0.5)
                                if kt >= qi * NF // P:
                                    # zero future keys (0/1 causal mask,
                                    # sliced from the 2-block mask tiles)
                                    j = kt - qi * NF // P
                                    nc.gpsimd.tensor_mul(
                                        at, at,
                                        cmask[j // 2][:, (j % 2) * NF:
                                                      (j % 2 + 1) * NF])
                                ats[h] = at
                            for h in pair:
                                kv = h // REP_
                                nc.tensor.matmul(
                                    otps[h],
                                    r(v1[kt][:, kv * (HD + 1):
                                             (kv + 1) * (HD + 1)]),
                                    r(ats[h]),
                                    start=(kt == 0), stop=(kt == nk - 1))
                        for h in pair:
                            otp = otps[h]
                            # rec sliced at base 64 so reciprocal's in/out
                            # APs sit on the same partition
                            rec = p3.tile([HD + 1, NF], F32, name="rec",
                                          tag="rec", bufs=2)
                            nc.vector.reciprocal(rec[HD:HD + 1, :],
                                                 otp[HD:HD + 1, :])
                            # broadcast across partitions via a DRAM hop
                            # with a stride-0 partition read (keeps GpSimd
                            # out of the chain)
                            recd = dp.tile([1, NF], F32, name="recd",
                                           tag="recd", bufs=3)
                            nc.sync.dma_start(out=recd,
                                              in_=rec[HD:HD + 1, :])
                            bc = p3.tile([HD, NF], F32, name="bc", tag="bc",
                                         bufs=2)
                            rb_bcast = bass.AP(
                                tensor=recd.tensor, offset=recd.offset,
                                ap=[[0, HD]] + list(recd.ap)[1:])
                            nc.sync.dma_start(out=bc, in_=rb_bcast)
                            ao = p3.tile([HD, NF], CDT, name="ao", tag="ao",
                                         bufs=3)
                            nc.vector.tensor_mul(ao, otp[0:HD, :], bc)
                            off = (h // NQT) * HD
                            nc.sync.dma_start(
                                out=ag_in[t][off:off + HD,
                                             qi * NF:(qi + 1) * NF],
                                in_=ao)
                        drip_wo(4)
                    drip_wo(len(wo_pending) if t == NQT - 1 else 0)
                    # pair tile t complete -> AllGather it while the next
                    # pair computes
                    nc.gpsimd.collective_compute(
                        "AllGather", mybir.AluOpType.bypass,
                        replica_groups=groups,
                        ins=[ag_in[t].opt()], outs=[ag_out[t].opt()])

            # ---------------- phase 5: out projection ----------------
            with tc.tile_pool(name="oppsum", bufs=6, space="PSUM") as pop:
                # rank-dynamic S-slice columns
                pid = nc.sync.partition_id()
                col0 = (pid % group) * OUT_S
                atf = {}
                for t in range(NQT):
                    for g in range(group):
                        a = p5.tile([P, OUT_S], MMDT, name=f"atf{t}_{g}")
                        nc.sync.dma_start(
                            out=a,
                            in_=ag_out[t][g * P:(g + 1) * P,
                                          ds(col0, OUT_S)])
                        atf[(t, g)] = a

                kts = [(t, g) for t in range(NQT) for g in range(group)]
                for n_o in range(WO_C // NF):
                    for m in range(OUT_S // P):
                        op = pop.tile([P, NF], F32, name="op", tag="op")
                        for kidx, (t, g) in enumerate(kts):
                            nc.tensor.matmul(
                                op,
                                r(atf[(t, g)][:, m * P:(m + 1) * P]),
                                r(wo_sb[(kidx, n_o)]),
                                start=(kidx == 0),
                                stop=(kidx == len(kts) - 1))
                        osb = p5.tile([P, NF], F32, name="osb", tag="osb",
                                      bufs=3)
                        nc.scalar.activation(osb, op, AF.Copy)
                        nc.sync.dma_start(
                            out=out_p[m * P:(m + 1) * P,
                                      n_o * NF:(n_o + 1) * NF],
                            in_=osb)
            p5_cm.__exit__(None, None, None)
            dram_pool.__exit__(None, None, None)
    nc.finalize()
    return nc


# ---------------------------------------------------------------------------
# host-side sharding / unsharding
# ---------------------------------------------------------------------------

def _local_head_perm(nhl):
    """Local head order: pairs (t, t + nhl//2)."""
    nqt = nhl // 2
    order = []
    for t in range(nqt):
        order.append(t)
        order.append(t + nqt)
    return order


def shard_inputs(x, Wq, Wk, Wv, Wo, S_=S, nhl=NHL, nkvl=NKVL, group=GROUP,
                 n_cores=N_CORES, mm="bf16"):
    """Build per-core input maps (converted to bf16 for mm="bf16")."""
    import ml_dtypes
    dt_ = ml_dtypes.bfloat16 if mm == "bf16" else np.float32
    perm = _local_head_perm(nhl)
    in_maps = []
    # Wo rows permuted identically for all cores. Order must match the
    # device-side gathered A^T layout: for each pair-tile t, the AllGather
    # concatenates the 4 ranks' 128-row chunks; within a chunk the rows are
    # heads (t, t+nqt) x 64.
    nqt = nhl // 2
    row_idx = []
    for t in range(nqt):
        for g in range(group):
            for h in (g * nhl + t, g * nhl + t + nqt):
                row_idx.extend(range(h * HD, (h + 1) * HD))
    wo_perm = np.ascontiguousarray(Wo[row_idx, :]).astype(dt_)
    for c in range(n_cores):
        b, rk = c // group, c % group
        col_idx = []
        for t in perm:
            h = rk * nhl + t
            col_idx.extend(range(h * HD, (h + 1) * HD))
        kv_cols = []
        for kvh in range(rk * nkvl, (rk + 1) * nkvl):
            kv_cols.extend(range(kvh * HD, (kvh + 1) * HD))
        in_maps.append({
            "x": np.ascontiguousarray(x[b]).astype(dt_),
            "wq": np.ascontiguousarray(Wq[:, col_idx]).astype(dt_),
            "wk": np.ascontiguousarray(Wk[:, kv_cols]).astype(dt_),
            "wv": np.ascontiguousarray(Wv[:, kv_cols]).astype(dt_),
            "wo": wo_perm,
        })
    return in_maps


def unshard_output(results, S_=S, group=GROUP, n_cores=N_CORES, width=DIM):
    out = np.zeros((n_cores // group, S_, width), dtype=np.float32)
    sl = S_ // group
    for c in range(n_cores):
        b, rk = c // group, c % group
        out[b, rk * sl:(rk + 1) * sl, :] = results[c]["out"]
    return out


_NC_CACHE = {}


def kernel(x, mask=None, Wq=None, Wk=None, Wv=None, Wo=None):
    """Full-input entry point: returns [B, S, DIM] float32."""
    global LAST_RESULTS
    from concourse.bass_utils import run_bass_kernel_spmd

    x = np.asarray(x, dtype=np.float32)
    key = "full"
    if key not in _NC_CACHE:
        _NC_CACHE[key] = build_nc()
    nc = _NC_CACHE[key]
    in_maps = shard_inputs(x, np.asarray(Wq), np.asarray(Wk),
                           np.asarray(Wv), np.asarray(Wo))
    res = run_bass_kernel_spmd(nc, in_maps, core_ids=list(range(N_CORES)),
                               trace=bool(os.environ.get("KERNEL_TRACE")))
    LAST_RESULTS = res
    return unshard_output(res.results)


# revision 39
# speedup vs baseline: 1.1420x; 1.0984x over previous
"""Distributed GQA attention kernel for 8 TRN2 NeuronCores.

Problem: B=2, S=2048, DIM=2048, NH=32 q heads, NKV=8 kv heads, HD=64,
RoPE (base 10000), causal mask, out-projection.

Sharding (8 cores): core c -> batch b = c//4, rank r = c%4.
Each core handles 8 q heads (r*8 .. r*8+7) and their 2 kv heads (2r, 2r+1)
for its batch:
  - q/k/v projections column-sharded (head dim), x transposed on-chip.
  - RoPE on qT/kT.
  - Flash-style causal attention in transposed layout (scoresT [k, q]),
    softmax without max-subtraction (scores are O(5) here), sums via an
    extra ones-column on v.
  - Four chunked AllGathers of A^T (one per q-head pair tile, bf16) over
    the 4-core batch group, overlapped with attention on later pairs.
  - Each core out-projects its own S-slice (rank-dynamic column slice via
    the partition_id input) with the full Wo -> out [512, 2048].
Host reassembles: out[b, r*512:(r+1)*512, :] = core (4*b + r) output.

Head pairing trick: local q heads are stored as pairs (t, t+4) in each
128-partition tile so that a head's partition base (0 or 64) always equals
its kv head's partition base -> matmul operand bases match (walrus
requirement). Wq columns and Wo rows are permuted host-side to compensate.
"""

import os
import numpy as np

import concourse.bass as bass
import concourse.mybir as mybir
from concourse import bacc, tile
from concourse.bass import ds

F32 = mybir.dt.float32
F32R = mybir.dt.float32r
AF = mybir.ActivationFunctionType

# -------- problem constants (full size) --------
B, S, DIM = 2, 2048, 2048
NH, NKV, HD = 32, 8, 64
ROPE_BASE = 10000.0
N_CORES = 8
GROUP = 4                      # cores per batch group
NHL = NH // GROUP              # 8 local q heads
NKVL = NKV // GROUP            # 2 local kv heads
REP = NH // NKV                # 4 q heads per kv head
P = 128                        # partitions
NF = 512                       # free-dim tile (one PSUM bank of f32)

# results of the last device run (for test harness introspection)
LAST_RESULTS = None


def _rope_tables(S_, dtype=np.float32):
    """cos table tiled to 128 partitions, and a sign-folded sin table:
    rows p with p%64 < 32 carry -sin (x1 half), else +sin (x2 half)."""
    inv_freq = 1.0 / (ROPE_BASE ** (np.arange(0, HD, 2, dtype=np.float64) / HD))
    t = np.arange(S_, dtype=np.float64)
    freqs = inv_freq[:, None] * t[None, :]          # [32, S]
    cos32 = np.cos(freqs).astype(dtype)
    sin32 = np.sin(freqs).astype(dtype)
    c128 = np.tile(cos32, (4, 1))                   # [128, S]
    s128 = np.tile(np.concatenate([-sin32, sin32], axis=0), (2, 1))
    return c128, s128


def build_nc(S_=S, DIM_=DIM, NHL_=NHL, NKVL_=NKVL, group=GROUP, n_cores=N_CORES,
             mm="bf16"):
    """Build the SPMD per-core graph. All cores run the identical graph;
    per-core behavior comes only from the input shards (+ partition_id).

    mm: matmul compute dtype: "bf16" (fast weight loads, 1 cyc/row),
    "f32r" (4-byte, 1 cyc/row at N>=256 but slow weight path), "f32".
    """
    NQT = NHL_ // 2                  # q-head tiles of 128 (pairs)
    REP_ = NHL_ // NKVL_             # q heads per kv head (local)
    assert NKVL_ == 2, "kT/v1 layout assumes 2 local kv heads"
    DQ = NHL_ * HD                   # local q width (512)
    DKV = NKVL_ * HD                 # 128
    n_chunks = S_ // NF              # S chunks for projections
    d_tiles = DIM_ // P              # contraction tiles for projections
    kt_total = S_ // P               # key tiles
    n_qt = S_ // NF                  # q tiles per head (free dim NF)
    OUT_S = S_ // group              # output S-slice rows per core
    WO_R = group * DQ                # Wo rows (= NH*HD globally)
    WO_C = DIM_                      # output width
    BF16 = mybir.dt.bfloat16
    MMDT = {"bf16": BF16, "f32r": F32R, "f32": F32}[mm]
    # dtype used on the x-transpose path (must match matmul dtype for bf16)
    TDT = BF16 if mm == "bf16" else F32
    # for bf16 the host pre-converts inputs -> plain HWDGE loads, no SWDGE
    # cast DMAs clogging GpSimd
    host_bf16 = mm == "bf16"
    IN_DT = BF16 if host_bf16 else F32
    cast_w = (MMDT != F32) and not host_bf16

    nc = bacc.Bacc(None, target_bir_lowering=False, num_devices=n_cores)

    x_p = nc.declare_dram_parameter("x", [S_, DIM_], IN_DT, isOutput=False)
    wq_p = nc.declare_dram_parameter("wq", [DIM_, DQ], IN_DT, isOutput=False)
    wk_p = nc.declare_dram_parameter("wk", [DIM_, DKV], IN_DT, isOutput=False)
    wv_p = nc.declare_dram_parameter("wv", [DIM_, DKV], IN_DT, isOutput=False)
    wo_p = nc.declare_dram_parameter("wo", [WO_R, WO_C], IN_DT, isOutput=False)
    out_p = nc.declare_dram_parameter("out", [OUT_S, WO_C], F32, isOutput=True)

    import ml_dtypes
    np_tdt = ml_dtypes.bfloat16 if TDT == BF16 else np.float32
    cos_np, sin_np = _rope_tables(S_)
    cos_d = nc.inline_tensor(cos_np, name="cos_tab")
    sin_d = nc.inline_tensor(sin_np, name="sin_tab")
    ident_d = nc.inline_tensor(np.eye(P, dtype=np_tdt), name="ident")
    ones_d = nc.inline_tensor(np.ones((P, 1), dtype=np_tdt), name="ones_col")
    # causal 0/1 mask tiles for the 4 diagonal-block offsets
    np_mmdt = {F32: np.float32, F32R: np.float32,
               BF16: ml_dtypes.bfloat16}[MMDT]
    xx = np.arange(P)[:, None]
    yy = np.arange(NF)[None, :]
    mask_np = np.stack([(yy - xx - j * P >= 0) for j in range(NF // P)])
    # pairs of diagonal blocks share one [128, 2*NF] mask
    mask2_np = np.stack([np.concatenate([mask_np[j], mask_np[j + 1]], axis=1)
                         for j in (0, 2)])
    mask_d = nc.inline_tensor(mask2_np.astype(np_mmdt), name="cmask")

    groups = [list(range(g * group, (g + 1) * group))
              for g in range(n_cores // group)]

    def r(ap):
        if ap.dtype == MMDT or MMDT == F32:
            return ap
        return ap.bitcast(MMDT)

    with tile.TileContext(nc) as tc:
        # ---------------- persistent tiles ----------------
        with tc.tile_pool(name="persist", bufs=1) as pp:
            ident = pp.tile([P, P], TDT, name="ident_sb")
            id_dma = nc.gpsimd if (TDT != F32 and not host_bf16) else nc.sync
            id_dma.dma_start(out=ident, in_=ident_d[:, :])
            cmask = [pp.tile([P, 2 * NF], MMDT, name=f"cmask{j}")
                     for j in range(2)]
            for j in range(2):
                id_dma.dma_start(out=cmask[j], in_=mask_d[j, :, :])

            qT = [pp.tile([P, S_], MMDT, name=f"qT{t}") for t in range(NQT)]
            kT = pp.tile([P, S_], MMDT, name="kT")
            # v1 tiles: [128 keys, 130]: cols 0-63 kv0, 64 ones, 65-128 kv1,
            # 129 ones
            v1 = [pp.tile([P, 2 * (HD + 1)], MMDT, name=f"v1_{c}")
                  for c in range(kt_total)]

            # ---------------- phase 1: x^T + projections + rope ---------
            with tc.tile_pool(name="p1", bufs=1) as p1, \
                 tc.tile_pool(name="p1psum", bufs=2, space="PSUM") as pp1, \
                 tc.tile_pool(name="accpsum", bufs=1, space="PSUM") as pacc:
                c128 = p1.tile([P, S_], F32, name="c128")
                s128 = p1.tile([P, S_], F32, name="s128")
                nc.scalar.dma_start(out=c128, in_=cos_d[:, :])
                nc.scalar.dma_start(out=s128, in_=sin_d[:, :])
                wq_sb = [p1.tile([P, DQ], MMDT, name=f"wq{k}")
                         for k in range(d_tiles)]
                wk_sb = [p1.tile([P, DKV], MMDT, name=f"wk{k}")
                         for k in range(d_tiles)]
                wv_sb = [p1.tile([P, DKV], MMDT, name=f"wv{k}")
                         for k in range(d_tiles)]
                wdma = nc.gpsimd if cast_w else nc.sync
                for k in range(d_tiles):
                    wdma.dma_start(out=wq_sb[k], in_=wq_p[k * P:(k + 1) * P, :])
                    wdma.dma_start(out=wk_sb[k], in_=wk_p[k * P:(k + 1) * P, :])
                    wdma.dma_start(out=wv_sb[k], in_=wv_p[k * P:(k + 1) * P, :])

                vT = p1.tile([P, S_], TDT, name="vT")
                NM = NQT + 2
                xdma = nc.gpsimd if (TDT != F32 and not host_bf16) else nc.sync

                for n in range(n_chunks):
                    ncol = slice(n * NF, (n + 1) * NF)
                    # x rows for this chunk: [NF, DIM] as NF//P tiles
                    x_rows = []
                    for st in range(NF // P):
                        xr = p1.tile([P, DIM_], TDT, name="xrow", tag="xrow",
                                     bufs=2 * (NF // P))
                        xdma.dma_start(
                            out=xr,
                            in_=x_p[n * NF + st * P: n * NF + (st + 1) * P, :])
                        x_rows.append(xr)
                    # k-outer: transpose one d-tile, feed all NM accumulators
                    accs = [pacc.tile([P, NF], F32, name=f"acc{m}",
                                      tag=f"acc{m}") for m in range(NM)]
                    for k in range(d_tiles):
                        tp = pp1.tile([P, NF], TDT, name="tp", tag="tp")
                        for st in range(NF // P):
                            nc.tensor.transpose(
                                tp[:, st * P:(st + 1) * P],
                                x_rows[st][:, k * P:(k + 1) * P],
                                ident)
                        xt = p1.tile([P, NF], MMDT, name="xt", tag="xt",
                                     bufs=4)
                        nc.scalar.activation(xt, tp, AF.Copy)
                        for m in range(NM):
                            if m < NQT:
                                lhsT = wq_sb[k][:, m * P:(m + 1) * P]
                            elif m == NQT:
                                lhsT = wk_sb[k]
                            else:
                                lhsT = wv_sb[k]
                            nc.tensor.matmul(accs[m], r(lhsT), r(xt),
                                             start=(k == 0),
                                             stop=(k == d_tiles - 1))
                    for m in range(NM):
                        acc = accs[m]
                        if m <= NQT:
                            # rope: dest qT[m] or kT.
                            # All DVE ops must be partition-aligned, so
                            # build a half-swapped copy via SBUF DMAs and
                            # use the sign-folded sin table:
                            #   out = raw*cos + swap(raw)*sgn_sin
                            dest = qT[m] if m < NQT else kT
                            qraw = p1.tile([P, NF], F32, name="qraw",
                                           tag="qraw", bufs=2)
                            nc.scalar.activation(qraw, acc, AF.Copy)
                            qswp = p1.tile([P, NF], F32, name="qswp",
                                           tag="qswp", bufs=2)
                            h32 = HD // 2
                            for blk in range(4):
                                src = blk ^ 1      # swap 32-blocks in pairs
                                nc.sync.dma_start(
                                    out=qswp[blk * h32:(blk + 1) * h32, :],
                                    in_=qraw[src * h32:(src + 1) * h32, :])
                            tcos = p1.tile([P, NF], F32, name="tcos",
                                           tag="tcos", bufs=2)
                            tsin = p1.tile([P, NF], F32, name="tsin",
                                           tag="tsin", bufs=2)
                            nc.vector.tensor_mul(tcos, qraw, c128[:, ncol])
                            nc.vector.tensor_mul(tsin, qswp, s128[:, ncol])
                            nc.vector.tensor_add(dest[:, ncol], tcos, tsin)
                        else:
                            nc.scalar.activation(vT[:, ncol], acc, AF.Copy)

                # ---- v transform: vT [128, S] -> v1 tiles [keys, 130] ----
                for c in range(kt_total):
                    tpv = pp1.tile([P, P], TDT, name="tpv", tag="tp")
                    nc.tensor.transpose(tpv, vT[:, c * P:(c + 1) * P], ident)
                    nc.vector.tensor_copy(v1[c][:, 0:HD], tpv[:, 0:HD])
                    nc.vector.tensor_copy(v1[c][:, HD + 1:2 * HD + 1],
                                          tpv[:, HD:2 * HD])
                    wdma.dma_start(out=v1[c][:, HD:HD + 1], in_=ones_d[:, :])
                    wdma.dma_start(out=v1[c][:, 2 * HD + 1:2 * HD + 2],
                                   in_=ones_d[:, :])

            # comm bounce buffers: one AllGather per q-head pair tile so the
            # collective for pair t overlaps attention on pair t+1
            CDT = BF16 if mm == "bf16" else F32
            dram_pool = tc.tile_pool(name="dram", bufs=1, space="DRAM")
            dp = dram_pool.__enter__()
            ag_in = [dp.tile([P, S_], CDT, name=f"ag_in{t}")
                     for t in range(NQT)]
            ag_out = [dp.tile([group * P, S_], CDT, name=f"ag_out{t}")
                      for t in range(NQT)]

            # Wo + A^T tiles for the out-projection (prefetched during
            # attention; everything fits easily in bf16)
            p5_cm = tc.tile_pool(name="p5", bufs=1)
            p5 = p5_cm.__enter__()
            wo_sb = {}
            wo_pending = []
            for kidx in range(WO_R // P):
                for n_o in range(WO_C // NF):
                    w = p5.tile([P, NF], MMDT, name=f"wo{kidx}_{n_o}")
                    wo_sb[(kidx, n_o)] = w
                    wo_pending.append((kidx, n_o))
            wo_dma = nc.gpsimd if cast_w else nc.sync

            def drip_wo(count):
                # spread the Wo prefetch so the DMA queue never backs up
                for _ in range(min(count, len(wo_pending))):
                    kidx, n_o = wo_pending.pop(0)
                    wo_dma.dma_start(
                        out=wo_sb[(kidx, n_o)],
                        in_=wo_p[kidx * P:(kidx + 1) * P,
                                 n_o * NF:(n_o + 1) * NF])

            # ---------------- phase 3: attention + chunked AllGather ----
            with tc.tile_pool(name="p3", bufs=1) as p3, \
                 tc.tile_pool(name="scpsum", bufs=4, space="PSUM") as psc, \
                 tc.tile_pool(name="otpsum", bufs=4, space="PSUM") as pot:
                for t in range(NQT):
                    pair = [t, t + NQT] if NQT < NHL_ else [t]
                    for qi in range(n_qt):
                        nk = (qi + 1) * NF // P
                        otps = {}
                        for h in pair:
                            otps[h] = pot.tile([HD + 1, NF], F32,
                                               name="otp", tag="otp")
                        for kt in range(nk):
                            # diagonal blocks: query columns y < 128*j are
                            # fully masked -- skip them in scores/exp/mask/PV
                            diag = kt >= qi * NF // P
                            j = kt - qi * NF // P if diag else 0
                            y0 = j * P if diag else 0
                            ats = {}
                            for h in pair:
                                kv = h // REP_
                                hb = (h // NQT) * HD
                                assert hb == kv * HD
                                sc = psc.tile([P, NF], F32, name="sc",
                                              tag="sc")
                                nc.tensor.matmul(
                                    sc[:, y0:],
                                    r(kT[hb:hb + HD, kt * P:(kt + 1) * P]),
                                    r(qT[t][hb:hb + HD,
                                            qi * NF + y0:(qi + 1) * NF]),
                                    start=True, stop=True)
                                at = p3.tile([P, NF], MMDT, name="at",
                                             tag="at", bufs=8)
                                nc.scalar.activation(at[:, y0:], sc[:, y0:],
                                                     AF.Exp,
                                                     scale=float(HD) ** -0.5)
                                if diag:
                                    # zero future keys (0/1 causal mask,
                                    # sliced from the 2-block mask tiles)
                                    nc.gpsimd.tensor_mul(
                                        at[:, y0:], at[:, y0:],
                                        cmask[j // 2][:, (j % 2) * NF + y0:
                                                      (j % 2 + 1) * NF])
                                ats[h] = at
                            for h in pair:
                                kv = h // REP_
                                nc.tensor.matmul(
                                    otps[h][:, y0:],
                                    r(v1[kt][:, kv * (HD + 1):
                                             (kv + 1) * (HD + 1)]),
                                    r(ats[h][:, y0:]),
                                    start=(kt == 0), stop=(kt == nk - 1))
                        for h in pair:
                            otp = otps[h]
                            # rec sliced at base 64 so reciprocal's in/out
                            # APs sit on the same partition
                            rec = p3.tile([HD + 1, NF], F32, name="rec",
                                          tag="rec", bufs=2)
                            nc.vector.reciprocal(rec[HD:HD + 1, :],
                                                 otp[HD:HD + 1, :])
                            # broadcast across partitions via a DRAM hop
                            # with a stride-0 partition read (keeps GpSimd
                            # out of the chain)
                            recd = dp.tile([1, NF], F32, name="recd",
                                           tag="recd", bufs=3)
                            nc.sync.dma_start(out=recd,
                                              in_=rec[HD:HD + 1, :])
                            bc = p3.tile([HD, NF], F32, name="bc", tag="bc",
                                         bufs=2)
                            rb_bcast = bass.AP(
                                tensor=recd.tensor, offset=recd.offset,
                                ap=[[0, HD]] + list(recd.ap)[1:])
                            nc.sync.dma_start(out=bc, in_=rb_bcast)
                            ao = p3.tile([HD, NF], CDT, name="ao", tag="ao",
                                         bufs=3)
                            nc.vector.tensor_mul(ao, otp[0:HD, :], bc)
                            off = (h // NQT) * HD
                            nc.sync.dma_start(
                                out=ag_in[t][off:off + HD,
                                             qi * NF:(qi + 1) * NF],
                                in_=ao)
                        drip_wo(4)
                    drip_wo(len(wo_pending) if t == NQT - 1 else 0)
                    # pair tile t complete -> AllGather it while the next
                    # pair computes
                    nc.gpsimd.collective_compute(
                        "AllGather", mybir.AluOpType.bypass,
                        replica_groups=groups,
                        ins=[ag_in[t].opt()], outs=[ag_out[t].opt()])

            # ---------------- phase 5: out projection ----------------
            with tc.tile_pool(name="oppsum", bufs=6, space="PSUM") as pop:
                # rank-dynamic S-slice columns
                pid = nc.sync.partition_id()
                col0 = (pid % group) * OUT_S
                atf = {}
                for t in range(NQT):
                    for g in range(group):
                        a = p5.tile([P, OUT_S], MMDT, name=f"atf{t}_{g}")
                        nc.sync.dma_start(
                            out=a,
                            in_=ag_out[t][g * P:(g + 1) * P,
                                          ds(col0, OUT_S)])
                        atf[(t, g)] = a

                kts = [(t, g) for t in range(NQT) for g in range(group)]
                for n_o in range(WO_C // NF):
                    for m in range(OUT_S // P):
                        op = pop.tile([P, NF], F32, name="op", tag="op")
                        for kidx, (t, g) in enumerate(kts):
                            nc.tensor.matmul(
                                op,
                                r(atf[(t, g)][:, m * P:(m + 1) * P]),
                                r(wo_sb[(kidx, n_o)]),
                                start=(kidx == 0),
                                stop=(kidx == len(kts) - 1))
                        osb = p5.tile([P, NF], F32, name="osb", tag="osb",
                                      bufs=3)
                        nc.scalar.activation(osb, op, AF.Copy)
                        nc.sync.dma_start(
                            out=out_p[m * P:(m + 1) * P,
                                      n_o * NF:(n_o + 1) * NF],
                            in_=osb)
            p5_cm.__exit__(None, None, None)
            dram_pool.__exit__(None, None, None)
    nc.finalize()
    return nc


# ---------------------------------------------------------------------------
# host-side sharding / unsharding
# ---------------------------------------------------------------------------

def _local_head_perm(nhl):
    """Local head order: pairs (t, t + nhl//2)."""
    nqt = nhl // 2
    order = []
    for t in range(nqt):
        order.append(t)
        order.append(t + nqt)
    return order


def shard_inputs(x, Wq, Wk, Wv, Wo, S_=S, nhl=NHL, nkvl=NKVL, group=GROUP,
                 n_cores=N_CORES, mm="bf16"):
    """Build per-core input maps (converted to bf16 for mm="bf16")."""
    import ml_dtypes
    dt_ = ml_dtypes.bfloat16 if mm == "bf16" else np.float32
    perm = _local_head_perm(nhl)
    in_maps = []
    # Wo rows permuted identically for all cores. Order must match the
    # device-side gathered A^T layout: for each pair-tile t, the AllGather
    # concatenates the 4 ranks' 128-row chunks; within a chunk the rows are
    # heads (t, t+nqt) x 64.
    nqt = nhl // 2
    row_idx = []
    for t in range(nqt):
        for g in range(group):
            for h in (g * nhl + t, g * nhl + t + nqt):
                row_idx.extend(range(h * HD, (h + 1) * HD))
    wo_perm = np.ascontiguousarray(Wo[row_idx, :]).astype(dt_)
    for c in range(n_cores):
        b, rk = c // group, c % group
        col_idx = []
        for t in perm:
            h = rk * nhl + t
            col_idx.extend(range(h * HD, (h + 1) * HD))
        kv_cols = []
        for kvh in range(rk * nkvl, (rk + 1) * nkvl):
            kv_cols.extend(range(kvh * HD, (kvh + 1) * HD))
        in_maps.append({
            "x": np.ascontiguousarray(x[b]).astype(dt_),
            "wq": np.ascontiguousarray(Wq[:, col_idx]).astype(dt_),
            "wk": np.ascontiguousarray(Wk[:, kv_cols]).astype(dt_),
            "wv": np.ascontiguousarray(Wv[:, kv_cols]).astype(dt_),
            "wo": wo_perm,
        })
    return in_maps


def unshard_output(results, S_=S, group=GROUP, n_cores=N_CORES, width=DIM):
    out = np.zeros((n_cores // group, S_, width), dtype=np.float32)
    sl = S_ // group
    for c in range(n_cores):
        b, rk = c // group, c % group
        out[b, rk * sl:(rk + 1) * sl, :] = results[c]["out"]
    return out


_NC_CACHE = {}


def kernel(x, mask=None, Wq=None, Wk=None, Wv=None, Wo=None):
    """Full-input entry point: returns [B, S, DIM] float32."""
    global LAST_RESULTS
    from concourse.bass_utils import run_bass_kernel_spmd

    x = np.asarray(x, dtype=np.float32)
    key = "full"
    if key not in _NC_CACHE:
        _NC_CACHE[key] = build_nc()
    nc = _NC_CACHE[key]
    in_maps = shard_inputs(x, np.asarray(Wq), np.asarray(Wk),
                           np.asarray(Wv), np.asarray(Wo))
    res = run_bass_kernel_spmd(nc, in_maps, core_ids=list(range(N_CORES)),
                               trace=bool(os.environ.get("KERNEL_TRACE")))
    LAST_RESULTS = res
    return unshard_output(res.results)
